# revision 89
# baseline (speedup 1.0000x reference)
"""Trainium2 Bass kernel: multi-head flash self-attention with RoPE.

Problem: x[4,2048,1024], 16 heads, dh=64, causal, RoPE(theta=10000), WO proj.

Sharding (8 cores): core c -> batch b=c//2, head-group g=c%2 (8 heads each).

v3 design notes (v2 + scheduling/engine-balance rework, 255.3us -> 243.5us):
  - Q/K/V projections in fp8e4m3 DoubleRow matmuls with a 3-term hi/lo
    split (x_hi*w_hi + x_lo*w_hi + x_hi*w_lo): bf16-level accuracy at
    0.75x the bf16 PE cost. Operands host-prepped in the DoubleRow
    plane-paired layout [d_part, 2, cols] (planes = D-slabs j, j+4),
    scaled by 2^5 (x) and 2^7 (w); the 2^-12 undo is folded into the
    RoPE tables (Q/K) and the V-evacuation copy.
  - Inputs are merged into a handful of big DMAs (x in one [p,hlj,plane,s]
    tensor loaded in 4 column phases, one tensor per (w, hi/lo), wo/cos/
    sin one each) ordered by first consumption — the shared HWDGE ring
    costs ~625ns per DMA regardless of size. cos/sin tables are bf16.
  - PE warmup: a few dummy matmuls on a memset tile keep the PE busy and
    p-state-ramped while the first input DMAs land.
  - Minimal eager prefix (V quarter 0 + q0/k0 only); the rest of the
    prefix is deadline-guarded filler (ensure_proj/ensure_v) pumped into
    p0's attention, overlapping the serialized input-DMA stream.
  - RoPE: psum is evacuated to bf16 SBUF first (frees the psum ring with
    no dependency on the table DMAs; Act engine does the copy outside
    attention, DVE inside), then mul/shuffle/add run in the DVE 2x bf16
    fast mode; sin is host-pre-swapped so the partition shuffle applies
    to the ps*sin product (one fewer psum-touching op).
  - Flash attention in S^T layout ([k,q] blocks), heads A/B fused: scores
    for both heads land in one 2-bank PSUM tile [128,1024]; ONE scalar-
    engine exp per k-block covers both heads (3D access pattern); the
    causal mask zeroes the diagonal q<k triangle AFTER the exp with one
    DVE fast-mode bf16 multiply by a 0/1 triangle tile (cheaper than PE
    mask matmuls, and off the Act critical path). V is stored per
    (ktile, head-pair) as [V_A | ones | V_B] so softmax denominators
    come out of the PV matmul for free. PV trails scores by three blocks
    (PSUM accumulation is order-independent) and independent matmuls
    (projections, output projection) are pumped into the PE stream
    between blocks to hide the exp latency.
  - PSUM pools are split by lifetime: scores (st, 2x2 banks), attention
    accumulators (att, 2 banks), everything else (ps5, 2 banks), so
    short-lived tiles never wait on the long-lived accumulators.
  - Per-pair AllGather of normalized O^T, deferred to mid-next-pair so
    its queue work misses the qb0 burst; pair 3 exchanged per-qb into
    per-qb tiles (no false whole-tile deps). Output projection is one
    fused 8-matmul psum accumulation per 128-row seq chunk, run as
    p3-attention filler as soon as its og3 slice lands; only the last
    qb group remains as tail.
"""
import sys

sys.path.insert(0, "/opt/trn_rl_repo")

import numpy as np
import ml_dtypes
import concourse.bass as bass
import concourse.bacc as bacc
import concourse.mybir as mybir
from concourse import tile
from concourse.bass_utils import run_bass_kernel_spmd

f32 = mybir.dt.float32
bf16 = mybir.dt.bfloat16
fp8 = mybir.dt.float8e4
AF = mybir.ActivationFunctionType
DR = mybir.MatmulPerfMode.DoubleRow
E4 = ml_dtypes.float8_e4m3

S = 2048
D = 1024
H = 16
DH = 64
NCORE = 8
SL = 512           # local m dims (8 heads x 64)
SCALE = 1.0 / 8.0  # 1/sqrt(dh)
GROUPS = [[0, 1], [2, 3], [4, 5], [6, 7]]
VPP = 192          # v_store cols per (ktile, pair): [V_A | ones | V_B]
VKT = 4 * VPP      # v_store cols per ktile
XSH = 5            # x quant scale 2^5
WSH = 7            # w quant scale 2^7
OSH = 5            # attention-output quant scale 2^5 (fp8 out projection)
UNDO = 2.0 ** (-(XSH + WSH))
SWAP_MASK = list(range(16, 32)) + list(range(16))  # rotate-half swap


def build(timing=False):
    nc = bacc.Bacc("TRN2", target_bir_lowering=False, debug=False,
                   num_devices=1 if timing else NCORE)

    # merged input tensors: one DMA each (HWDGE overhead is per-DMA)
    x8 = nc.dram_tensor("x8all", [128, 8 * 2 * S], fp8,
                        kind="ExternalInput").ap()      # [p, hlj, plane, s]
    w8 = {}
    for w in ("wq", "wk", "wv"):
        for hl in "hl":
            w8[w, hl] = nc.dram_tensor(f"{w}8{hl}", [128, 4 * 1024], fp8,
                                       kind="ExternalInput").ap()
    woT = nc.dram_tensor("woT", [128, 8 * SL], bf16,
                         kind="ExternalInput").ap()     # [p, dt, m]
    cosr = nc.dram_tensor("cosr", [128, S], bf16, kind="ExternalInput").ap()
    sinr = nc.dram_tensor("sinr", [128, S], bf16, kind="ExternalInput").ap()
    out = nc.dram_tensor("out", [S, SL], bf16, kind="ExternalOutput").ap()

    og_send = [nc.dram_tensor(f"og_send{p}", [128, S], bf16) for p in range(3)]
    og_recv = [nc.dram_tensor(f"og_recv{p}", [256, S], bf16) for p in range(3)]
    # pair 3 exchanges per-qb; collectives need contiguous dram patterns
    og_send.append([nc.dram_tensor(f"og_send3_{qb}", [128, 512], bf16)
                    for qb in range(4)])
    og_recv.append([nc.dram_tensor(f"og_recv3_{qb}", [256, 512], bf16)
                    for qb in range(4)])

    with tile.TileContext(nc) as tc:
        _body(nc, tc, x8, w8, woT, cosr, sinr, out, og_send, og_recv, timing)
    nc.compile()
    return nc


def _body(nc, tc, x8, w8, woT, cosr, sinr, out, og_send, og_recv,
          timing=False):
    from contextlib import ExitStack
    ctx = ExitStack()
    with ctx:
        sb = ctx.enter_context(tc.tile_pool(name="sb", bufs=1))
        psp = ctx.enter_context(tc.tile_pool(name="psp", bufs=1, space="PSUM"))
        counter = [0]

        def til(shape, dtype, tag, bufs):
            counter[0] += 1
            return sb.tile(shape, dtype, tag=tag, bufs=bufs,
                           name=f"{tag}_{counter[0]}")

        # ---------------- input loads, consumption order ----------------
        # x lives in one big tile [p, hlj(8), plane(2), s(2048)]; weights in
        # one tile per (w, hl) viewed per j. Loads are merged into few big
        # DMAs (the shared HWDGE ring costs ~625ns per DMA regardless of
        # size) and ordered to match first consumption.
        xt_all = None   # assigned below
        wtil = {}       # (w, hl) -> [128, 4, 1024] view source tile

        def x_3d(hl, j):
            a = (0 if hl == "h" else 1) * 4 + j
            return xt_all[:].rearrange(
                "p (a j s) -> p a j s", a=8, j=2)[:, a]

        def w_view(w, hl, j):
            return wtil[w, hl][:].rearrange(
                "p (j m) -> p j m", j=4)[:, j]

        def x8_4d():
            return x8[:].rearrange("p (a j s) -> p a j s", a=8, j=2)

        def load_x_phase(ph, half=None):
            cols = slice(ph * 512, (ph + 1) * 512)
            sl = slice(0, 8) if half is None else (
                slice(0, 4) if half == "h" else slice(4, 8))
            nc.sync.dma_start(
                xt_all[:].rearrange("p (a j s) -> p a j s", a=8, j=2)
                [:, sl, :, cols],
                x8_4d()[:, sl, :, cols])

        # gpsimd constants first so they don't queue behind SWDGE loads.
        # warm_t feeds PE warmup matmuls that keep the PE busy (and ramp its
        # p-state) while the first input DMAs land.
        warm_t = til([128, 512], bf16, "warm", 1)
        nc.vector.memset(warm_t[:], 0.5)
        # causal-mask triangle (1 where q >= k, else 0), duplicated for the
        # two fused heads: the diagonal block of the exp output is multiplied
        # by this on the DVE (bf16 fast mode) instead of adding -1e30 on the
        # PE before the exp.
        tri_t = til([128, 256], bf16, "tri", 1)
        nc.gpsimd.memset(tri_t[:], 1.0)
        nc.gpsimd.affine_select(
            out=tri_t[:].rearrange("p (j c) -> p j c", j=2),
            in_=tri_t[:].rearrange("p (j c) -> p j c", j=2),
            compare_op=mybir.AluOpType.is_ge,
            fill=0.0, base=0, pattern=[[0, 2], [1, 128]],
            channel_multiplier=-1,
        )
        # v quarters: v_q[i] holds ktiles 4i..4i+4; per (kt, pair p) block
        # of VPP cols: [V_A | ones | V_B]
        v_q = []
        for i in range(16):
            vq = til([128, VKT], bf16, "v", 16)
            nc.gpsimd.memset(vq[:], 1.0)
            v_q.append(vq)

        # PE warmup: dummy matmuls with no data dependencies so the PE is
        # busy (and fully p-state ramped) by the time the first x/wv DMAs
        # land. The psum tile cycles through the shared ps5 pool.
        warm_ps = psp.tile([128, 512], f32, tag="ps5", bufs=2, name="warm_ps")
        for _ in range(7):
            nc.tensor.matmul(warm_ps[:], warm_t[:, 0:128], warm_t[:],
                             start=True, stop=True)

        xt_all = til([128, 8 * 2 * S], fp8, "x8", 1)
        for w in ("wv", "wq", "wk"):
            for hl in "hl":
                wtil[w, hl] = til([128, 4 * 1024], fp8, w, 2)
        # consumption order: V quarter 0 (wv-h, x0-h, wv-l, x0-l), q0 proj
        # (wq), rope (cos/sin), k0 proj (wk), V quarters / proj st 1-3
        # (x phases), output projection (wo)
        nc.sync.dma_start(wtil["wv", "h"][:], w8["wv", "h"][:])
        nc.sync.dma_start(
            xt_all[:].rearrange("p (a j s) -> p a j s", a=8, j=2)
            [:, 0:4, :, 0:512], x8_4d()[:, 0:4, :, 0:512])
        nc.sync.dma_start(wtil["wv", "l"][:], w8["wv", "l"][:])
        nc.sync.dma_start(
            xt_all[:].rearrange("p (a j s) -> p a j s", a=8, j=2)
            [:, 4:8, :, 0:512], x8_4d()[:, 4:8, :, 0:512])
        nc.sync.dma_start(wtil["wq", "h"][:], w8["wq", "h"][:])
        nc.sync.dma_start(wtil["wq", "l"][:], w8["wq", "l"][:])
        cos_t = til([128, S], bf16, "cos", 1)
        sin_t = til([128, S], bf16, "sin", 1)
        nc.sync.dma_start(sin_t[:], sinr[:])
        nc.sync.dma_start(cos_t[:], cosr[:])
        nc.sync.dma_start(wtil["wk", "h"][:], w8["wk", "h"][:])
        nc.sync.dma_start(wtil["wk", "l"][:], w8["wk", "l"][:])
        load_x_phase(1, "h")
        load_x_phase(1, "l")
        load_x_phase(2, "h")
        load_x_phase(2, "l")
        load_x_phase(3, "h")
        load_x_phase(3, "l")
        wt_all = til([128, 8 * SL], bf16, "wo", 1)
        nc.sync.dma_start(wt_all[:], woT[:])
        wt = [wt_all[:].rearrange("p (dt m) -> p dt m", dt=8)[:, dt]
              for dt in range(8)]

        def proj_mms(ps, stat_of, mov_of):
            """12 DoubleRow matmuls, term-major so they track DMA arrival."""
            terms = [("h", "h"), ("l", "h"), ("h", "l")]
            n = 0
            for (a, b) in terms:
                for j in range(4):
                    n += 1
                    nc.tensor.matmul(
                        ps, stat_of(a, j), mov_of(b, j),
                        start=(n == 1), stop=(n == 12),
                        perf_mode=DR,
                    )

        v_done = [False] * 16

        def emit_v_quarter(i, eager=True):
            for kt4 in range(4):
                kt = 4 * i + kt4
                cell = [None]

                def mk(kt, cell, n, a, b, j):
                    def thunk():
                        if n == 0:
                            counter[0] += 1
                            cell[0] = psp.tile([128, 512], f32,
                                               tag="ps5", bufs=2,
                                               name=f"vps_{counter[0]}")
                        nc.tensor.matmul(
                            cell[0][:],
                            x_3d(a, j)[:, :, kt * 128:(kt + 1) * 128],
                            w_view("wv", b, j).rearrange(
                                "p (j m) -> p j m", j=2),
                            start=(n == 0), stop=(n == 11), perf_mode=DR)
                        if n == 11:
                            vva = v_q[kt][:].rearrange(
                                "q (a c) -> q a c", c=64)
                            psa = cell[0][:].rearrange(
                                "q (a c) -> q a c", c=64)
                            nc.vector.tensor_scalar_mul(
                                vva[:, 0:12:3, :], psa[:, 0:8:2, :], UNDO)
                            nc.vector.tensor_scalar_mul(
                                vva[:, 2:12:3, :], psa[:, 1:8:2, :], UNDO)
                            v_done[kt] = True
                    return thunk

                terms = [("h", "h"), ("h", "l"), ("l", "h")]
                for n, (a, b, j) in enumerate(
                        (a, b, j) for (a, b) in terms for j in range(4)):
                    t = mk(kt, cell, n, a, b, j)
                    if eager:
                        t()
                    else:
                        filler.append(t)
                if eager:
                    v_done[kt] = True

        def ensure_v(kt):
            while not v_done[kt]:
                filler.popleft()()

        def v_slice(kt, p, c0, c1):
            off = p * VPP
            return v_q[kt][:, off + c0:off + c1]

        from collections import deque
        filler = deque()   # single-MM thunks of attention-independent work

        def pump(n):
            k = 0
            while filler and k < n:
                filler.popleft()()
                k += 1

        def flush_filler():
            while filler:
                filler.popleft()()

        proj_done = {}   # (wname, mt, st) -> True once fully emitted
        in_attention = [False]   # rope evac engine: Act when idle, else DVE

        def ensure_proj(wname, mt, st):
            while not proj_done.get((wname, mt, st)):
                filler.popleft()()

        # per-st projection + rope into a [128, 512] bf16 tile. When
        # eager=False the 12 matmuls are enqueued as filler thunks; the
        # rope chain is emitted by the last thunk.
        def proj_rope_st(wname, mt, st, eager=True):
            big_t = til([128, 512], bf16, "qk", 17)
            ps = psp.tile([128, 512], f32, tag="ps5", bufs=2)

            def stat(a, j):
                return w_view(wname, a, j).rearrange(
                    "p (j m) -> p j m", j=2)[:, :, mt * 128:(mt + 1) * 128]

            def mov(b, j):
                return x_3d(b, j)[:, :, st * 512:(st + 1) * 512]

            def rope():
                # rope chain: evacuate the psum to bf16 SBUF first (frees the
                # psum ring without waiting on the cos/sin table DMAs), then
                # run the muls in the DVE 2x bf16 fast mode. sin is host-pre-
                # swapped so the partition shuffle applies to the ps*sin
                # product. Prefix (eager) evacs ride the idle Act engine;
                # filler evacs stay on DVE to protect the exp pacing.
                cols = slice(st * 512, (st + 1) * 512)
                qf = til([128, 512], bf16, "qf", 3)
                nc.scalar.activation(qf[:], ps[:], AF.Copy)
                tmp = til([128, 512], bf16, "tmp", 2)
                nc.vector.tensor_mul(tmp[:], qf[:], cos_t[:, cols])
                u = til([128, 512], bf16, "swp", 2)
                nc.vector.tensor_mul(u[:], qf[:], sin_t[:, cols])
                swp2 = til([128, 512], bf16, "swp2", 2)
                nc.vector.stream_shuffle(swp2[:], u[:], SWAP_MASK)
                nc.vector.tensor_add(big_t[:], tmp[:], swp2[:])

            terms = [("h", "h"), ("l", "h"), ("h", "l")]
            steps = [(n, a, b, j) for n, (a, b, j) in enumerate(
                (a, b, j) for (a, b) in terms for j in range(4))]

            def mk(n, a, b, j):
                def thunk():
                    nc.tensor.matmul(ps[:], stat(a, j), mov(b, j),
                                     start=(n == 0), stop=(n == 11),
                                     perf_mode=DR)
                    if n == 11:
                        rope()
                        proj_done[wname, mt, st] = True
                return thunk

            for (n, a, b, j) in steps:
                t = mk(n, a, b, j)
                if eager:
                    t()
                else:
                    filler.append(t)
            return big_t

        ofull = [[None, None] for _ in range(4)]   # [pair<3][member]
        ofull3 = [[None] * 4, [None] * 4]          # [member][qb] (pair 3)

        def exchange_pair(p, qb=None):
            """AllGather pair p's O^T (whole pair, or one qb slice)."""
            if qb is None:
                snd, rcv = og_send[p][:], og_recv[p]
            else:
                snd, rcv = og_send[3][qb][:], og_recv[3][qb]
            if timing:
                # stub the AllGather as two gpsimd-queue (SWDGE) copies,
                # mirroring the real collective's Pool-engine placement
                nc.gpsimd.dma_start(rcv[0:128, :].opt(), snd.opt())
                nc.gpsimd.dma_start(rcv[128:256, :].opt(), snd.opt())
            else:
                nc.gpsimd.collective_compute(
                    "AllGather", mybir.AluOpType.bypass,
                    replica_groups=GROUPS,
                    ins=[snd.opt()], outs=[rcv[:].opt()],
                )
            for g2 in range(2):
                if qb is None:
                    ofull[p][g2] = til([128, S], bf16, "of", 6)
                    dst = ofull[p][g2][:]
                else:
                    # per-qb tiles so later out_st16 reads don't pick up
                    # false whole-tile deps on future qb writes
                    ofull3[g2][qb] = til([128, 512], bf16, "of3", 8)
                    dst = ofull3[g2][qb][:]
                nc.sync.dma_start(
                    dst, rcv[g2 * 128:(g2 + 1) * 128, :].opt())

        def o_slab(p, g2, st16):
            if p < 3:
                return ofull[p][g2][:, st16 * 128:(st16 + 1) * 128]
            return ofull3[g2][st16 // 4][:, (st16 % 4) * 128:
                                         (st16 % 4 + 1) * 128]

        # -------- per pair: Q/K projection + rope + flash attention --------

        def attention_qb(p, qb, qtr, ktr):
            if p == 0:
                ensure_proj("wq", 0, qb)
                ensure_proj("wk", 0, qb)
            qcols_t = qtr[qb]
            oA = psp.tile([128, 512], f32, tag="att", bufs=2)
            oB = psp.tile([128, 512], f32, tag="att", bufs=2)
            nkb = 4 * (qb + 1)

            def emit_scores(kb):
                if p == 0:
                    # p0's k tiles may still be in the filler queue
                    ensure_proj("wk", 0, kb // 4)
                kt_t = ktr[kb // 4]
                kcols = slice((kb % 4) * 128, (kb % 4) * 128 + 128)
                jrel = kb - 4 * qb
                lo = max(jrel, 0) * 128   # first valid q col in block
                sub = slice(lo, 512)
                stAB = psp.tile([128, 1024], f32, tag="st", bufs=2)
                diag = jrel >= 0
                nc.tensor.matmul(stAB[:, lo:512], kt_t[0:64, kcols],
                                 qcols_t[0:64, sub],
                                 start=True, stop=True)
                nc.tensor.matmul(stAB[:, 512 + lo:1024],
                                 kt_t[64:128, kcols],
                                 qcols_t[64:128, sub],
                                 start=True, stop=True)
                pAB = til([128, 1024], bf16, "p", 6)
                st3 = stAB[:].rearrange("p (j c) -> p j c", j=2)
                p3 = pAB[:].rearrange("p (j c) -> p j c", j=2)
                nc.scalar.activation(p3[:, :, sub], st3[:, :, sub],
                                     AF.Exp, scale=SCALE)
                if diag:
                    # causal mask: zero the q<k triangle of the diagonal
                    # 128-col sub-block of both heads with one fast-mode
                    # bf16 multiply on the DVE
                    nc.vector.tensor_mul(
                        p3[:, :, lo:lo + 128], p3[:, :, lo:lo + 128],
                        tri_t[:].rearrange("p (j c) -> p j c", j=2))
                return pAB, lo, sub

            def emit_pv(kb, pAB, lo, sub):
                nc.tensor.matmul(oA[:, sub], v_slice(kb, p, 0, 128),
                                 pAB[:, sub],
                                 start=(kb == 0), stop=(kb == nkb - 1))
                nc.tensor.matmul(oB[:, sub], v_slice(kb, p, 64, 192),
                                 pAB[:, 512 + lo:1024],
                                 start=(kb == 0), stop=(kb == nkb - 1))

            # three-block software pipeline: PV(kb) trails scores(kb+3);
            # PSUM accumulation is order-independent so this is safe, and
            # stAB is freed by the exp, not the PV. The extra depth hides
            # the Pool affine_select latency on diagonal blocks.
            rate = 4 if p == 3 else (3 if qb == 3 else 2)
            depth = 3
            pend = []
            in_attention[0] = True
            h1 = rate // 2
            for kb in range(nkb):
                pend.append((kb,) + emit_scores(kb))
                pump(h1)
                if len(pend) > depth:
                    e = pend.pop(0)
                    if p == 0:
                        ensure_v(e[0])
                    emit_pv(*e)
                pump(rate - h1)
            while pend:
                pump(h1)
                e = pend.pop(0)
                if p == 0:
                    ensure_v(e[0])
                emit_pv(*e)
                pump(rate - h1)
            in_attention[0] = False
            # normalize. A psum rows: [O_A | l_A]; B psum rows: [l_B | O_B]
            qcols = slice(qb * 512, (qb + 1) * 512)
            onrm = til([128, 512], bf16, "onrm", 4)
            rc = til([128, 512], f32, "rc", 2)
            nc.vector.reciprocal(rc[64:128, :], oA[64:128, :])
            nc.vector.reciprocal(rc[0:64, :], oB[0:64, :])
            rc2 = til([128, 512], f32, "rc2", 2)
            nc.sync.dma_start(rc2[0:64, :], rc[64:128, :])
            nc.sync.dma_start(rc2[64:128, :], rc[0:64, :])
            nc.vector.tensor_mul(onrm[0:64, :], oA[0:64, :], rc2[0:64, :])
            nc.vector.tensor_mul(onrm[64:128, :], oB[64:128, :],
                                 rc2[64:128, :])
            if p == 3:
                nc.sync.dma_start(og_send[3][qb][:].opt(), onrm[:])
                exchange_pair(3, qb)
            else:
                nc.sync.dma_start(og_send[p][:, qcols].opt(), onrm[:])

        # output projection: one fused psum accumulation per st16 (seq chunk
        # of 128 q rows) over all 8 (pair, member) d-slabs, one evacuation
        # op on the Act engine, one direct [128, 512] store. Runs as
        # p3-attention filler once the corresponding og3 qb exchange landed.
        DTS = [(p, g2) for g2 in range(2) for p in range(4)]

        def out_st16(st16, eager=False):
            cell = [None]
            thunks = []
            ptag = "att" if eager else "ps5"
            for i, (p, g2) in enumerate(DTS):
                def mk(i, p, g2):
                    def thunk():
                        if i == 0:
                            counter[0] += 1
                            cell[0] = psp.tile([128, 512], f32,
                                               tag=ptag, bufs=2,
                                               name=f"ops_{counter[0]}")
                        nc.tensor.matmul(
                            cell[0][:], o_slab(p, g2, st16),
                            wt[4 * g2 + p],
                            start=(i == 0), stop=(i == len(DTS) - 1),
                        )
                        if i == len(DTS) - 1:
                            ofin = til([128, SL], bf16, "ofin", 4)
                            nc.vector.tensor_copy(ofin[:], cell[0][:])
                            nc.sync.dma_start(
                                out[st16 * 128:(st16 + 1) * 128, :], ofin[:])
                    return thunk
                t = mk(i, p, g2)
                if eager:
                    t()
                else:
                    thunks.append(t)
            return thunks

        # ---------------- schedule ----------------
        # p0 prefix: V quarter 0 + p0's Q/K proj+rope eager; V quarters
        # 1-3 go to the filler queue (deadline-guarded by ensure_v)
        # minimal eager prefix: V quarter 0 + q0/k0, then p0's attention
        # starts; the rest of the prefix (q/k st 1-3, V quarters 1-3) is
        # deadline-guarded filler pumped during the attention, so the PE
        # overlaps the serialized input-DMA stream instead of stalling on it
        trq = {0: [None] * 4}
        trk = {0: [None] * 4}
        emit_v_quarter(0)
        trq[0][0] = proj_rope_st("wq", 0, 0)
        trk[0][0] = proj_rope_st("wk", 0, 0)
        for st in range(1, 4):
            trq[0][st] = proj_rope_st("wq", 0, st, eager=False)
            trk[0][st] = proj_rope_st("wk", 0, st, eager=False)
            emit_v_quarter(st, eager=False)

        for p in range(4):
            if p < 3:
                # next pair's Q/K projections as attention filler,
                # interleaved q0,k0,q1,k1,... so early tiles finish first
                trq[p + 1] = [None] * 4
                trk[p + 1] = [None] * 4
                for st in range(4):
                    trq[p + 1][st] = proj_rope_st("wq", p + 1, st,
                                                  eager=False)
                    trk[p + 1][st] = proj_rope_st("wk", p + 1, st,
                                                  eager=False)
            for qb in range(4):
                attention_qb(p, qb, trq[p], trk[p])
                if qb == 0 and p > 0:
                    # deferred to mid-pair so the previous pair's exchange
                    # doesn't collide with qb0's work
                    exchange_pair(p - 1)
                if p == 3:
                    # st16 group qb is fully determined (its og3 slice just
                    # exchanged at the end of qb) — fused output projection
                    # as filler for the remaining attention
                    for st16 in range(4 * qb, 4 * qb + 4):
                        filler.extend(out_st16(st16))
            flush_filler()
